# revision 10
# baseline (speedup 1.0000x reference)
"""Trainium2 Bass kernel: BiLSTM(2-layer, bidir) -> GraphConv -> mean-pool
-> tanh(rep1-rep2) @ pred_w + pred_b, data-parallel over graphs on 8 cores.

kernel(**inputs) takes FULL unsharded inputs (as in reference.setup_inputs())
and returns the FULL [2048, 2] float32 output.

Device layout (per core: 256 graphs, 8960 tokens, hardcoded):
  Everything is feature-major transposed [feat, token] with t-major token
  columns (token (t,g) at column t*256+g), so an LSTM step is a contiguous
  256-column slice.  Gates for step t live in one PSUM tile [128, 8, 256]
  (slot order i,i,f,f,o,o,g,g after host-side permutation of the 4H weight
  columns); the input projection, the recurrent matmul and the bias all
  accumulate there (L1 bias rides as an extra K-row of x; L2 bias is a K=1
  matmul against a ones row).  One sigmoid over slots 0:6 and one tanh over
  slots 6:8 then feed the DVE c/h updates; h is written straight into the
  layer-output tile in bf16.  GCN: z = D_in^-1/2 A D_out^-1/2 (o W) + b as
  (1) a projection matmul whose stationary operand gathers one 3-graph block
  of columns, (2) a block-diagonal augmented-adjacency matmul (bias folded as
  a 106th contraction row), then relu, mean-pool matmul, and the prediction
  head.  Matmul operands bf16 (host casts), fp32 accumulation everywhere.
"""

import os
import sys

for _p in ("/opt/trn_rl_repo", "/root/.axon_site/_ro/trn_rl_repo"):
    if _p not in sys.path and os.path.isdir(_p):
        sys.path.insert(0, _p)

import numpy as np
import ml_dtypes

BF16 = ml_dtypes.bfloat16

G = 2048
NPG = 35
DEG = 8
INPUT = 300
HL = 256
HG = 256
OUT = 2
NCORES = 8
GC = G // NCORES          # 256 graphs / core
B = GC * NPG              # 8960 tokens / core
H4 = 4 * HL               # 1024
NBLK = 86                 # 3-graph blocks (last block: graphs 253-255, only
                          # graph 255 live in the adjacency)

_CACHE = {}


def _build_bass(reps=1):
    import concourse.tile as tile
    from concourse import bacc, mybir

    f32 = mybir.dt.float32
    bf16 = mybir.dt.bfloat16
    AF = mybir.ActivationFunctionType

    nc = bacc.Bacc("TRN2", target_bir_lowering=False, debug=False,
                   num_devices=NCORES)

    dram = {}
    for br in (1, 2):
        dram[f"xT{br}"] = nc.dram_tensor(f"xT{br}", [INPUT + 1, B], bf16,
                                         kind="ExternalInput")
        for d in (0, 1):
            dram[f"w1ih{br}{d}"] = nc.dram_tensor(
                f"w1ih{br}{d}", [INPUT + 1, H4], bf16, kind="ExternalInput")
            dram[f"w1hh{br}{d}"] = nc.dram_tensor(
                f"w1hh{br}{d}", [HL, H4], bf16, kind="ExternalInput")
            dram[f"w2ih{br}{d}"] = nc.dram_tensor(
                f"w2ih{br}{d}", [2 * HL, H4], bf16, kind="ExternalInput")
            dram[f"w2hh{br}{d}"] = nc.dram_tensor(
                f"w2hh{br}{d}", [HL, H4], bf16, kind="ExternalInput")
            dram[f"b2_{br}{d}"] = nc.dram_tensor(
                f"b2_{br}{d}", [1, H4], bf16, kind="ExternalInput")
        dram[f"adjT{br}"] = nc.dram_tensor(
            f"adjT{br}", [105, NBLK * 105], bf16, kind="ExternalInput")
        dram[f"gw{br}"] = nc.dram_tensor(
            f"gw{br}", [2 * HL, HG], bf16, kind="ExternalInput")
        dram[f"gb{br}"] = nc.dram_tensor(
            f"gb{br}", [105, HG], bf16, kind="ExternalInput")
    dram["predw"] = nc.dram_tensor("predw", [HG, OUT], bf16,
                                   kind="ExternalInput")
    dram["predb"] = nc.dram_tensor("predb", [OUT, 1], f32,
                                   kind="ExternalInput")
    dram["pool3"] = nc.dram_tensor("pool3", [105, 3], bf16,
                                   kind="ExternalInput")
    out_dram = nc.dram_tensor("outT", [OUT, GC], f32, kind="ExternalOutput")

    with tile.TileContext(nc) as tc:
        with (
            tc.tile_pool(name="consts", bufs=1) as consts,
            tc.tile_pool(name="opool", bufs=1) as opool,
            tc.tile_pool(name="state", bufs=1) as state,
            tc.tile_pool(name="spool", bufs=2) as spool,
            tc.tile_pool(name="rpool", bufs=1) as rpool,
        ):
            ones_row = consts.tile([1, GC], bf16)
            nc.vector.memset(ones_row, 1.0)
            pool3 = consts.tile([105, 3], bf16)
            nc.sync.dma_start(out=pool3, in_=dram["pool3"][:, :])
            predw = consts.tile([128, 2, OUT], bf16)
            nc.sync.dma_start(
                out=predw,
                in_=dram["predw"].rearrange("(c p) o -> p c o", p=128))
            predb = consts.tile([OUT, 1], f32)
            nc.sync.dma_start(out=predb, in_=dram["predb"][:, :])

            repS = {br: [rpool.tile([128, GC], f32, tag=f"rep{br}_{h}", name=f"rep{br}_{h}")
                         for h in (0, 1)] for br in (1, 2)}

            def lstm_layer(layer, wih, whh, b2, rhs_src, outT, gpsum):
                nihc = len(wih[0])
                cst = {d: state.tile([128, 2, GC], f32, tag=f"c{d}", name=f"c{d}")
                       for d in (0, 1)}
                for k in range(NPG):
                    for d in (0, 1):
                        t = k if d == 0 else NPG - 1 - k
                        tp = t - 1 if d == 0 else t + 1
                        gates = gpsum.tile([128, 8, GC], f32, tag=f"g{d}")
                        for m in range(8):
                            msl = slice(m * 128, (m + 1) * 128)
                            ops = []
                            if b2 is not None:
                                ops.append((b2[d][0:1, msl], ones_row[0:1, :]))
                            for kc in range(nihc):
                                ops.append((wih[d][kc][:, msl],
                                            rhs_src[kc][:, t, :]))
                            if k > 0:
                                for kc in range(2):
                                    ops.append((whh[d][kc][:, msl],
                                                outT[2 * d + kc][:, tp, :]))
                            last = len(ops) - 1
                            for i, (lhsT, rhs) in enumerate(ops):
                                nc.tensor.matmul(gates[:, m, :], lhsT, rhs,
                                                 start=(i == 0),
                                                 stop=(i == last))
                        S = spool.tile([128, 8, GC], bf16, tag="S")
                        nc.scalar.activation(S[:, 0:6, :], gates[:, 0:6, :],
                                             AF.Sigmoid)
                        nc.scalar.activation(S[:, 6:8, :], gates[:, 6:8, :],
                                             AF.Tanh)
                        c_ = cst[d]
                        if k == 0:
                            nc.vector.tensor_mul(c_[:, :, :], S[:, 0:2, :],
                                                 S[:, 6:8, :])
                        else:
                            tmp = spool.tile([128, 2, GC], f32, tag="tmp")
                            nc.vector.tensor_mul(tmp, S[:, 0:2, :],
                                                 S[:, 6:8, :])
                            nc.vector.tensor_mul(c_[:, :, :], c_[:, :, :],
                                                 S[:, 2:4, :])
                            nc.vector.tensor_add(c_[:, :, :], c_[:, :, :],
                                                 tmp)
                        tch = spool.tile([128, 2, GC], bf16, tag="tc")
                        nc.scalar.activation(tch, c_[:, :, :], AF.Tanh)
                        for j in (0, 1):
                            nc.vector.tensor_mul(outT[2 * d + j][:, t, :],
                                                 S[:, 4 + j, :],
                                                 tch[:, j, :])

            for rep in range(reps):
                for br in (1, 2):
                    with (
                        tc.tile_pool(name=f"x{br}_{rep}", bufs=1) as xpool,
                        tc.tile_pool(name=f"ps{br}_{rep}", bufs=1,
                                     space="PSUM") as gpsum,
                    ):
                        xT = []
                        for kc in range(3):
                            p0 = kc * 128
                            pn = min(128, INPUT + 1 - p0)
                            t_ = xpool.tile([pn, NPG, GC], bf16,
                                            tag=f"xT{kc}")
                            nc.sync.dma_start(
                                out=t_,
                                in_=dram[f"xT{br}"][p0:p0 + pn, :].rearrange(
                                    "p (t g) -> p t g", g=GC))
                            xT.append(t_)
                        o1T = [opool.tile([128, NPG, GC], bf16, tag=f"o1T{c}",
                                          name=f"o1T{c}") for c in range(4)]

                        # ---- layer 1 ----
                        with tc.tile_pool(name=f"w1_{br}_{rep}",
                                          bufs=1) as wp:
                            w1ih, w1hh = {}, {}
                            for d in (0, 1):
                                w1ih[d] = []
                                for kc in range(3):
                                    p0 = kc * 128
                                    pn = min(128, INPUT + 1 - p0)
                                    t_ = wp.tile([pn, H4], bf16,
                                                 tag=f"w1ih{d}{kc}")
                                    nc.sync.dma_start(
                                        out=t_,
                                        in_=dram[f"w1ih{br}{d}"]
                                        [p0:p0 + pn, :])
                                    w1ih[d].append(t_)
                                w1hh[d] = []
                                for kc in range(2):
                                    t_ = wp.tile([128, H4], bf16,
                                                 tag=f"w1hh{d}{kc}")
                                    nc.sync.dma_start(
                                        out=t_,
                                        in_=dram[f"w1hh{br}{d}"]
                                        [kc * 128:(kc + 1) * 128, :])
                                    w1hh[d].append(t_)
                            lstm_layer(1, w1ih, w1hh, None, xT, o1T, gpsum)

                    # ---- layer 2 (xT freed) ----
                    with tc.tile_pool(name=f"o2_{br}_{rep}", bufs=1) as o2p:
                        o2T = [o2p.tile([128, NPG, GC], bf16, tag=f"o2T{c}",
                                        name=f"o2T{c}") for c in range(4)]
                        with (
                            tc.tile_pool(name=f"w2_{br}_{rep}",
                                         bufs=1) as wp2,
                            tc.tile_pool(name=f"ps2{br}_{rep}", bufs=1,
                                         space="PSUM") as gpsum2,
                        ):
                            w2ih, w2hh, b2 = {}, {}, {}
                            for d in (0, 1):
                                w2ih[d] = []
                                for kc in range(4):
                                    t_ = wp2.tile([128, H4], bf16,
                                                  tag=f"w2ih{d}{kc}")
                                    nc.sync.dma_start(
                                        out=t_,
                                        in_=dram[f"w2ih{br}{d}"]
                                        [kc * 128:(kc + 1) * 128, :])
                                    w2ih[d].append(t_)
                                w2hh[d] = []
                                for kc in range(2):
                                    t_ = wp2.tile([128, H4], bf16,
                                                  tag=f"w2hh{d}{kc}")
                                    nc.sync.dma_start(
                                        out=t_,
                                        in_=dram[f"w2hh{br}{d}"]
                                        [kc * 128:(kc + 1) * 128, :])
                                    w2hh[d].append(t_)
                                b2[d] = wp2.tile([1, H4], bf16, tag=f"b2{d}", name=f"b2{d}")
                                nc.sync.dma_start(
                                    out=b2[d], in_=dram[f"b2_{br}{d}"][:, :])
                            lstm_layer(2, w2ih, w2hh, b2, o1T, o2T, gpsum2)

                        # ---- GCN + mean pool (w2 freed, o2T alive) ----
                        with (
                            tc.tile_pool(name=f"gc{br}_{rep}", bufs=3) as gp,
                            tc.tile_pool(name=f"ga{br}_{rep}", bufs=2) as gap,
                            tc.tile_pool(name=f"gps{br}_{rep}", bufs=2,
                                         space="PSUM") as pps,
                            tc.tile_pool(name=f"zps{br}_{rep}", bufs=2,
                                         space="PSUM") as zps,
                            tc.tile_pool(name=f"rps{br}_{rep}", bufs=1,
                                         space="PSUM") as rps,
                        ):
                            gw = gp.tile([128, 4, HG], bf16, tag="gw")
                            nc.sync.dma_start(
                                out=gw,
                                in_=dram[f"gw{br}"].rearrange(
                                    "(c p) o -> p c o", p=128))
                            gb = gp.tile([105, HG], bf16, tag="gb")
                            nc.sync.dma_start(out=gb, in_=dram[f"gb{br}"][:, :])
                            repP = [rps.tile([128, GC], f32, tag=f"repP{h}",
                                             name=f"repP{h}") for h in (0, 1)]
                            ZBG = 4
                            adj_dram = dram[f"adjT{br}"].rearrange(
                                "k (b m) -> k b m", m=105)
                            for bg in range(0, NBLK, ZBG):
                                nb = min(ZBG, NBLK - bg)
                                adj = gap.tile([105, ZBG, 105], bf16,
                                               tag="adj")
                                nc.sync.dma_start(
                                    out=adj[:, 0:nb, :],
                                    in_=adj_dram[:, bg:bg + nb, :])
                                zp = zps.tile([105, ZBG, HG], f32, tag="zp")
                                zr = gp.tile([105, ZBG, HG], bf16, tag="zr")
                                for bi in range(nb):
                                    blk = bg + bi
                                    g0 = blk * 3 if blk < NBLK - 1 else GC - 3
                                    pp = pps.tile([105, HG], f32, tag="pp")
                                    # matmul operands need a single free dim:
                                    # gather the block's 105 columns into a
                                    # contiguous stage via idle GpSimd
                                    stg = gp.tile([128, 4, 105], bf16,
                                                  tag="stg")
                                    for kc in range(4):
                                        nc.gpsimd.tensor_copy(
                                            stg[:, kc, :].rearrange(
                                                "p (t g) -> p t g", g=3),
                                            o2T[kc][:, :, g0:g0 + 3])
                                    for kc in range(4):
                                        nc.tensor.matmul(
                                            pp, stg[:, kc, :], gw[:, kc, :],
                                            start=(kc == 0), stop=(kc == 3))
                                    pb = gp.tile([105, HG], bf16, tag="pb")
                                    nc.vector.tensor_copy(pb, pp[0:105, :])
                                    nc.tensor.matmul(zp[:, bi, :],
                                                     adj[:, bi, :], pb,
                                                     start=True, stop=True)
                                import concourse.bass as bass
                                gbb = bass.AP(tensor=gb.tensor,
                                              offset=gb.offset,
                                              ap=[list(gb.ap[0]), [0, nb],
                                                  list(gb.ap[1])])
                                nc.vector.tensor_add(zp[:, 0:nb, :],
                                                     zp[:, 0:nb, :], gbb)
                                nc.scalar.activation(zr[:, 0:nb, :],
                                                     zp[:, 0:nb, :], AF.Relu)
                                for bi in range(nb):
                                    blk = bg + bi
                                    for h in (0, 1):
                                        lhsT = zr[:, bi,
                                                  h * 128:(h + 1) * 128]
                                        if blk < NBLK - 1:
                                            nc.tensor.matmul(
                                                repP[h][:, blk * 3:
                                                        blk * 3 + 3],
                                                lhsT, pool3,
                                                start=True, stop=True)
                                        else:
                                            nc.tensor.matmul(
                                                repP[h][:, GC - 1:GC],
                                                lhsT, pool3[:, 2:3],
                                                start=True, stop=True)
                            for h in (0, 1):
                                nc.vector.tensor_copy(repS[br][h], repP[h])

                # ---- head ----
                with (
                    tc.tile_pool(name=f"hd_{rep}", bufs=1) as hp,
                    tc.tile_pool(name=f"hps_{rep}", bufs=1,
                                 space="PSUM") as hps,
                ):
                    dist = [hp.tile([128, GC], bf16, tag=f"dist{h}",
                                     name=f"dist{h}") for h in (0, 1)]
                    for h in (0, 1):
                        dsub = hp.tile([128, GC], f32, tag=f"dsub{h}")
                        nc.vector.tensor_sub(dsub, repS[1][h], repS[2][h])
                        nc.scalar.activation(dist[h], dsub, AF.Tanh)
                    op = hps.tile([OUT, GC], f32, tag="op")
                    for h in (0, 1):
                        nc.tensor.matmul(op, predw[:, h, :], dist[h],
                                         start=(h == 0), stop=(h == 1))
                    oS = hp.tile([OUT, GC], f32, tag="oS")
                    nc.scalar.activation(oS, op, AF.Identity, bias=predb)
                    nc.sync.dma_start(out=out_dram[:, :], in_=oS)

    nc.finalize()
    return nc


# ----------------------------------------------------------------------------
# Host-side preparation
# ----------------------------------------------------------------------------

def _perm_gates():
    """Columns: pytorch gate order [i,f,g,o] -> slots [i0,i1,f0,f1,o0,o1,g0,g1]."""
    idx = np.arange(H4).reshape(4, 2, 128)
    order = [(0, 0), (0, 1), (1, 0), (1, 1), (3, 0), (3, 1), (2, 0), (2, 1)]
    return np.concatenate([idx[g, h] for g, h in order])


def _prep_lstm(lstm, perm):
    out = {}
    for layer in range(2):
        for d in range(2):
            Wih, Whh, bih, bhh = [np.asarray(a) for a in lstm[layer][d]]
            bias = (bih + bhh)[perm]
            WihT = Wih.T[:, perm]
            WhhT = Whh.T[:, perm]
            if layer == 0:
                WihT = np.concatenate([WihT, bias[None, :]], 0)
                out[f"w1ih{d}"] = np.ascontiguousarray(WihT).astype(BF16)
                out[f"w1hh{d}"] = np.ascontiguousarray(WhhT).astype(BF16)
            else:
                out[f"w2ih{d}"] = np.ascontiguousarray(WihT).astype(BF16)
                out[f"w2hh{d}"] = np.ascontiguousarray(WhhT).astype(BF16)
                out[f"b2_{d}"] = bias[None, :].astype(BF16)
    return out


def _prep_adj(src, dst):
    src = np.asarray(src).astype(np.int64)
    dst = np.asarray(dst).astype(np.int64)
    g = dst // NPG
    sl = src - g * NPG
    dl = dst - g * NPG
    flat = (g * NPG + dl) * NPG + sl
    A = np.bincount(flat, minlength=G * NPG * NPG).reshape(G, NPG, NPG)
    outdeg = np.maximum(A.sum(axis=1), 1.0)
    Ahat = A / np.sqrt(8.0) / np.sqrt(outdeg)[:, None, :]
    return Ahat.astype(np.float32)


def _pack_adj_core(Ahat_c):
    """[GC,35,35] -> [106, NBLK*105] bf16 lhsT blocks with bias-ones row.

    lhsT block layout: row k = source node in the projection's M order, which
    is t-major (k = t_src*3 + i for graph position i), plus row 105 = bias
    ones; col m = destination node position-major (m = i*35 + t_dst), to
    match the pooling matrix.  Block NBLK-1 covers graphs GC-3..GC-1 but only
    position 2 (graph GC-1) is live."""
    out = np.zeros((105, NBLK * 105), np.float32)
    for blk in range(NBLK):
        if blk < NBLK - 1:
            g0, live = blk * 3, (0, 1, 2)
        else:
            g0, live = GC - 3, (2,)
        for i in live:
            c0 = blk * 105 + i * NPG
            # rows t_src*3 + i, cols t_dst: Ahat[t_dst, t_src].T = [src, dst]
            out[i:i + 3 * NPG:3, c0:c0 + NPG] = Ahat_c[g0 + i].T
    return out.astype(BF16)


def _prep_host(x1, x2, src1, dst1, src2, dst2, lstm1, lstm2,
               gcn1_w, gcn1_b, gcn2_w, gcn2_b, pred_w, pred_b):
    perm = _perm_gates()
    branches = {}
    for br, (x, lstm, gw_, gb_, src, dst) in enumerate(
            [(x1, lstm1, gcn1_w, gcn1_b, src1, dst1),
             (x2, lstm2, gcn2_w, gcn2_b, src2, dst2)], start=1):
        branches[br] = (np.asarray(x, np.float32), _prep_lstm(lstm, perm),
                        np.asarray(gw_), np.asarray(gb_),
                        _prep_adj(src, dst))

    pool3 = np.zeros((105, 3), np.float32)
    for i in range(3):
        pool3[i * NPG:(i + 1) * NPG, i] = 1.0 / NPG

    in_maps = []
    for c in range(NCORES):
        m = {}
        for br in (1, 2):
            x, lw, gw_, gb_, Ahat = branches[br]
            xs = x[c * B:(c + 1) * B]
            xT = np.empty((INPUT + 1, B), np.float32)
            xT[:INPUT] = xs.reshape(GC, NPG, INPUT).transpose(2, 1, 0) \
                           .reshape(INPUT, B)
            xT[INPUT] = 1.0
            m[f"xT{br}"] = xT.astype(BF16)
            for d in (0, 1):
                for k in ("w1ih", "w1hh", "w2ih", "w2hh"):
                    m[f"{k}{br}{d}"] = lw[f"{k}{d}"]
                m[f"b2_{br}{d}"] = lw[f"b2_{d}"]
            m[f"adjT{br}"] = _pack_adj_core(Ahat[c * GC:(c + 1) * GC])
            m[f"gw{br}"] = gw_.astype(BF16)
            m[f"gb{br}"] = np.broadcast_to(
            gb_.reshape(1, HG), (105, HG)).astype(BF16)
        m["predw"] = np.asarray(pred_w).astype(BF16)
        m["predb"] = np.asarray(pred_b).reshape(OUT, 1).astype(np.float32)
        m["pool3"] = pool3.astype(BF16)
        in_maps.append(m)
    return in_maps


def kernel(**inputs):
    from concourse.bass_utils import run_bass_kernel_spmd

    in_maps = _prep_host(**inputs)
    if "nc" not in _CACHE:
        _CACHE["nc"] = _build_bass()
    res = run_bass_kernel_spmd(_CACHE["nc"], in_maps,
                               core_ids=list(range(NCORES)))
    out = np.empty((G, OUT), np.float32)
    for c in range(NCORES):
        out[c * GC:(c + 1) * GC] = res.results[c]["outT"].T
    return out


# revision 12
# speedup vs baseline: 1.0661x; 1.0661x over previous
"""Trainium2 Bass kernel: BiLSTM(2-layer, bidir) -> GraphConv -> mean-pool
-> tanh(rep1-rep2) @ pred_w + pred_b, data-parallel over graphs on 8 cores.

kernel(**inputs) takes FULL unsharded inputs (as in reference.setup_inputs())
and returns the FULL [2048, 2] float32 output.

Device layout (per core: 256 graphs, 8960 tokens, hardcoded):
  Everything is feature-major transposed [feat, token] with t-major token
  columns (token (t,g) at column t*256+g), so an LSTM step is a contiguous
  256-column slice.  Gates for step t live in one PSUM tile [128, 8, 256]
  (slot order i,i,f,f,o,o,g,g after host-side permutation of the 4H weight
  columns); the input projection, the recurrent matmul and the bias all
  accumulate there (L1 bias rides as an extra K-row of x; L2 bias is a K=1
  matmul against a ones row).  One sigmoid over slots 0:6 and one tanh over
  slots 6:8 then feed the DVE c/h updates; h is written straight into the
  layer-output tile in bf16.  GCN: z = D_in^-1/2 A D_out^-1/2 (o W) + b as
  (1) a projection matmul whose stationary operand gathers one 3-graph block
  of columns, (2) a block-diagonal augmented-adjacency matmul (bias folded as
  a 106th contraction row), then relu, mean-pool matmul, and the prediction
  head.  Matmul operands bf16 (host casts), fp32 accumulation everywhere.
"""

import os
import sys

for _p in ("/opt/trn_rl_repo", "/root/.axon_site/_ro/trn_rl_repo"):
    if _p not in sys.path and os.path.isdir(_p):
        sys.path.insert(0, _p)

import numpy as np
import ml_dtypes

BF16 = ml_dtypes.bfloat16

G = 2048
NPG = 35
DEG = 8
INPUT = 300
HL = 256
HG = 256
OUT = 2
NCORES = 8
GC = G // NCORES          # 256 graphs / core
B = GC * NPG              # 8960 tokens / core
H4 = 4 * HL               # 1024
NBLK = 86                 # 3-graph blocks (last block: graphs 253-255, only
                          # graph 255 live in the adjacency)

_CACHE = {}


def _build_bass(reps=1):
    import concourse.tile as tile
    from concourse import bacc, mybir

    f32 = mybir.dt.float32
    bf16 = mybir.dt.bfloat16
    AF = mybir.ActivationFunctionType

    nc = bacc.Bacc("TRN2", target_bir_lowering=False, debug=False,
                   num_devices=NCORES)

    dram = {}
    for br in (1, 2):
        dram[f"xT{br}"] = nc.dram_tensor(f"xT{br}", [INPUT + 1, B], bf16,
                                         kind="ExternalInput")
        for d in (0, 1):
            dram[f"w1ih{br}{d}"] = nc.dram_tensor(
                f"w1ih{br}{d}", [INPUT + 1, H4], bf16, kind="ExternalInput")
            dram[f"w1hh{br}{d}"] = nc.dram_tensor(
                f"w1hh{br}{d}", [HL, H4], bf16, kind="ExternalInput")
            dram[f"w2ih{br}{d}"] = nc.dram_tensor(
                f"w2ih{br}{d}", [2 * HL, H4], bf16, kind="ExternalInput")
            dram[f"w2hh{br}{d}"] = nc.dram_tensor(
                f"w2hh{br}{d}", [HL, H4], bf16, kind="ExternalInput")
            dram[f"b2_{br}{d}"] = nc.dram_tensor(
                f"b2_{br}{d}", [1, H4], bf16, kind="ExternalInput")
        dram[f"adjT{br}"] = nc.dram_tensor(
            f"adjT{br}", [105, NBLK * 105], bf16, kind="ExternalInput")
        dram[f"gw{br}"] = nc.dram_tensor(
            f"gw{br}", [2 * HL, HG], bf16, kind="ExternalInput")
        dram[f"gb{br}"] = nc.dram_tensor(
            f"gb{br}", [105, HG], bf16, kind="ExternalInput")
    dram["predw"] = nc.dram_tensor("predw", [HG, OUT], bf16,
                                   kind="ExternalInput")
    dram["predb"] = nc.dram_tensor("predb", [OUT, 1], f32,
                                   kind="ExternalInput")
    dram["pool3"] = nc.dram_tensor("pool3", [105, 3], bf16,
                                   kind="ExternalInput")
    out_dram = nc.dram_tensor("outT", [OUT, GC], f32, kind="ExternalOutput")

    with tile.TileContext(nc) as tc:
        with (
            tc.tile_pool(name="consts", bufs=1) as consts,
            tc.tile_pool(name="opool", bufs=1) as opool,
            tc.tile_pool(name="state", bufs=1) as state,
            tc.tile_pool(name="spool", bufs=2) as spool,
            tc.tile_pool(name="rpool", bufs=1) as rpool,
        ):
            ones_row = consts.tile([1, GC], bf16)
            nc.vector.memset(ones_row, 1.0)
            pool3 = consts.tile([105, 3], bf16)
            nc.sync.dma_start(out=pool3, in_=dram["pool3"][:, :])
            predw = consts.tile([128, 2, OUT], bf16)
            nc.sync.dma_start(
                out=predw,
                in_=dram["predw"].rearrange("(c p) o -> p c o", p=128))
            predb = consts.tile([OUT, 1], f32)
            nc.sync.dma_start(out=predb, in_=dram["predb"][:, :])

            repS = {br: [rpool.tile([128, GC], f32, tag=f"rep{br}_{h}", name=f"rep{br}_{h}")
                         for h in (0, 1)] for br in (1, 2)}

            def lstm_layer(layer, wih, whh, b2, rhs_src, outT, gpsum):
                # rhs_src: list of [128, NPG, GC] K-chunk tiles (3 or 4+1)
                # outT: dict d -> [128, 2, NPG, GC] per-direction output tile
                nihc = len(wih[0])
                cst = {d: state.tile([128, 2, GC], f32, tag=f"c{d}", name=f"c{d}")
                       for d in (0, 1)}
                for k in range(NPG):
                    Ss = {}
                    # phase 1: both chains' matmuls + gate activations, so
                    # the ACT stream never stalls behind the DVE c-chain
                    for d in (0, 1):
                        t = k if d == 0 else NPG - 1 - k
                        tp = t - 1 if d == 0 else t + 1
                        gates = gpsum.tile([128, 8, GC], f32, tag=f"g{d}")
                        for m in range(8):
                            msl = slice(m * 128, (m + 1) * 128)
                            ops = []
                            if b2 is not None:
                                ops.append((b2[d][0:1, msl], ones_row[0:1, :]))
                            for kc in range(nihc):
                                ops.append((wih[d][kc][:, msl],
                                            rhs_src[kc][:, t, :]))
                            if k > 0:
                                for kc in range(2):
                                    ops.append((whh[d][kc][:, msl],
                                                outT[d][:, kc, tp, :]))
                            last = len(ops) - 1
                            for i, (lhsT, rhs) in enumerate(ops):
                                nc.tensor.matmul(gates[:, m, :], lhsT, rhs,
                                                 start=(i == 0),
                                                 stop=(i == last))
                        S = spool.tile([128, 8, GC], bf16, tag="S")
                        nc.scalar.activation(S[:, 0:6, :], gates[:, 0:6, :],
                                             AF.Sigmoid)
                        nc.scalar.activation(S[:, 6:8, :], gates[:, 6:8, :],
                                             AF.Tanh)
                        Ss[d] = S
                    # phase 2: c update (DVE), tanh(c) (ACT), h write (DVE)
                    for d in (0, 1):
                        t = k if d == 0 else NPG - 1 - k
                        S = Ss[d]
                        c_ = cst[d]
                        if k == 0:
                            nc.vector.tensor_mul(c_[:, :, :], S[:, 0:2, :],
                                                 S[:, 6:8, :])
                        else:
                            tmp = spool.tile([128, 2, GC], f32, tag="tmp")
                            nc.vector.tensor_mul(tmp, S[:, 0:2, :],
                                                 S[:, 6:8, :])
                            nc.vector.tensor_mul(c_[:, :, :], c_[:, :, :],
                                                 S[:, 2:4, :])
                            nc.vector.tensor_add(c_[:, :, :], c_[:, :, :],
                                                 tmp)
                        tch = spool.tile([128, 2, GC], bf16, tag="tc")
                        nc.scalar.activation(tch, c_[:, :, :], AF.Tanh)
                        nc.vector.tensor_mul(outT[d][:, :, t, :],
                                             S[:, 4:6, :], tch)

            for rep in range(reps):
                for br in (1, 2):
                    with (
                        tc.tile_pool(name=f"x{br}_{rep}", bufs=1) as xpool,
                        tc.tile_pool(name=f"ps{br}_{rep}", bufs=1,
                                     space="PSUM") as gpsum,
                    ):
                        xT = []
                        for kc in range(3):
                            p0 = kc * 128
                            pn = min(128, INPUT + 1 - p0)
                            t_ = xpool.tile([pn, NPG, GC], bf16,
                                            tag=f"xT{kc}")
                            nc.sync.dma_start(
                                out=t_,
                                in_=dram[f"xT{br}"][p0:p0 + pn, :].rearrange(
                                    "p (t g) -> p t g", g=GC))
                            xT.append(t_)
                        o1T = {d: opool.tile([128, 2, NPG, GC], bf16,
                                             tag=f"o1T{d}", name=f"o1T{d}")
                               for d in (0, 1)}

                        # ---- layer 1 ----
                        with tc.tile_pool(name=f"w1_{br}_{rep}",
                                          bufs=1) as wp:
                            w1ih, w1hh = {}, {}
                            for d in (0, 1):
                                w1ih[d] = []
                                for kc in range(3):
                                    p0 = kc * 128
                                    pn = min(128, INPUT + 1 - p0)
                                    t_ = wp.tile([pn, H4], bf16,
                                                 tag=f"w1ih{d}{kc}")
                                    nc.sync.dma_start(
                                        out=t_,
                                        in_=dram[f"w1ih{br}{d}"]
                                        [p0:p0 + pn, :])
                                    w1ih[d].append(t_)
                                w1hh[d] = []
                                for kc in range(2):
                                    t_ = wp.tile([128, H4], bf16,
                                                 tag=f"w1hh{d}{kc}")
                                    nc.sync.dma_start(
                                        out=t_,
                                        in_=dram[f"w1hh{br}{d}"]
                                        [kc * 128:(kc + 1) * 128, :])
                                    w1hh[d].append(t_)
                            lstm_layer(1, w1ih, w1hh, None, xT, o1T, gpsum)

                    # ---- layer 2 (xT freed) ----
                    with tc.tile_pool(name=f"o2_{br}_{rep}", bufs=1) as o2p:
                        o2T = {d: o2p.tile([128, 2, NPG, GC], bf16,
                                           tag=f"o2T{d}", name=f"o2T{d}")
                               for d in (0, 1)}
                        with (
                            tc.tile_pool(name=f"w2_{br}_{rep}",
                                         bufs=1) as wp2,
                            tc.tile_pool(name=f"ps2{br}_{rep}", bufs=1,
                                         space="PSUM") as gpsum2,
                        ):
                            w2ih, w2hh, b2 = {}, {}, {}
                            for d in (0, 1):
                                w2ih[d] = []
                                for kc in range(4):
                                    t_ = wp2.tile([128, H4], bf16,
                                                  tag=f"w2ih{d}{kc}")
                                    nc.sync.dma_start(
                                        out=t_,
                                        in_=dram[f"w2ih{br}{d}"]
                                        [kc * 128:(kc + 1) * 128, :])
                                    w2ih[d].append(t_)
                                w2hh[d] = []
                                for kc in range(2):
                                    t_ = wp2.tile([128, H4], bf16,
                                                  tag=f"w2hh{d}{kc}")
                                    nc.sync.dma_start(
                                        out=t_,
                                        in_=dram[f"w2hh{br}{d}"]
                                        [kc * 128:(kc + 1) * 128, :])
                                    w2hh[d].append(t_)
                                b2[d] = wp2.tile([1, H4], bf16, tag=f"b2{d}", name=f"b2{d}")
                                nc.sync.dma_start(
                                    out=b2[d], in_=dram[f"b2_{br}{d}"][:, :])
                            o1v = [o1T[0][:, 0], o1T[0][:, 1],
                                   o1T[1][:, 0], o1T[1][:, 1]]
                            lstm_layer(2, w2ih, w2hh, b2, o1v, o2T, gpsum2)

                        # ---- GCN + mean pool (w2 freed, o2T alive) ----
                        with (
                            tc.tile_pool(name=f"gc{br}_{rep}", bufs=3) as gp,
                            tc.tile_pool(name=f"ga{br}_{rep}", bufs=2) as gap,
                            tc.tile_pool(name=f"gps{br}_{rep}", bufs=2,
                                         space="PSUM") as pps,
                            tc.tile_pool(name=f"zps{br}_{rep}", bufs=2,
                                         space="PSUM") as zps,
                            tc.tile_pool(name=f"rps{br}_{rep}", bufs=1,
                                         space="PSUM") as rps,
                        ):
                            gw = gp.tile([128, 4, HG], bf16, tag="gw")
                            nc.sync.dma_start(
                                out=gw,
                                in_=dram[f"gw{br}"].rearrange(
                                    "(c p) o -> p c o", p=128))
                            gb = gp.tile([105, HG], bf16, tag="gb")
                            nc.sync.dma_start(out=gb, in_=dram[f"gb{br}"][:, :])
                            repP = [rps.tile([128, GC], f32, tag=f"repP{h}",
                                             name=f"repP{h}") for h in (0, 1)]
                            ZBG = 4
                            adj_dram = dram[f"adjT{br}"].rearrange(
                                "k (b m) -> k b m", m=105)
                            for bg in range(0, NBLK, ZBG):
                                nb = min(ZBG, NBLK - bg)
                                adj = gap.tile([105, ZBG, 105], bf16,
                                               tag="adj")
                                nc.sync.dma_start(
                                    out=adj[:, 0:nb, :],
                                    in_=adj_dram[:, bg:bg + nb, :])
                                zp = zps.tile([105, ZBG, HG], f32, tag="zp")
                                zr = gp.tile([105, ZBG, HG], bf16, tag="zr")
                                for bi in range(nb):
                                    blk = bg + bi
                                    g0 = blk * 3 if blk < NBLK - 1 else GC - 3
                                    pp = pps.tile([105, HG], f32, tag="pp")
                                    # matmul operands need a single free dim:
                                    # gather the block's 105 columns into a
                                    # contiguous stage via idle GpSimd
                                    stg = gp.tile([128, 4, 105], bf16,
                                                  tag="stg")
                                    o2v = [o2T[0][:, 0], o2T[0][:, 1],
                                           o2T[1][:, 0], o2T[1][:, 1]]
                                    for kc in range(4):
                                        nc.gpsimd.tensor_copy(
                                            stg[:, kc, :].rearrange(
                                                "p (t g) -> p t g", g=3),
                                            o2v[kc][:, :, g0:g0 + 3])
                                    for kc in range(4):
                                        nc.tensor.matmul(
                                            pp, stg[:, kc, :], gw[:, kc, :],
                                            start=(kc == 0), stop=(kc == 3))
                                    pb = gp.tile([105, HG], bf16, tag="pb")
                                    nc.vector.tensor_copy(pb, pp[0:105, :])
                                    nc.tensor.matmul(zp[:, bi, :],
                                                     adj[:, bi, :], pb,
                                                     start=True, stop=True)
                                import concourse.bass as bass
                                gbb = bass.AP(tensor=gb.tensor,
                                              offset=gb.offset,
                                              ap=[list(gb.ap[0]), [0, nb],
                                                  list(gb.ap[1])])
                                nc.vector.tensor_add(zp[:, 0:nb, :],
                                                     zp[:, 0:nb, :], gbb)
                                nc.scalar.activation(zr[:, 0:nb, :],
                                                     zp[:, 0:nb, :], AF.Relu)
                                for bi in range(nb):
                                    blk = bg + bi
                                    for h in (0, 1):
                                        lhsT = zr[:, bi,
                                                  h * 128:(h + 1) * 128]
                                        if blk < NBLK - 1:
                                            nc.tensor.matmul(
                                                repP[h][:, blk * 3:
                                                        blk * 3 + 3],
                                                lhsT, pool3,
                                                start=True, stop=True)
                                        else:
                                            nc.tensor.matmul(
                                                repP[h][:, GC - 1:GC],
                                                lhsT, pool3[:, 2:3],
                                                start=True, stop=True)
                            for h in (0, 1):
                                nc.vector.tensor_copy(repS[br][h], repP[h])

                # ---- head ----
                with (
                    tc.tile_pool(name=f"hd_{rep}", bufs=1) as hp,
                    tc.tile_pool(name=f"hps_{rep}", bufs=1,
                                 space="PSUM") as hps,
                ):
                    dist = [hp.tile([128, GC], bf16, tag=f"dist{h}",
                                     name=f"dist{h}") for h in (0, 1)]
                    for h in (0, 1):
                        dsub = hp.tile([128, GC], f32, tag=f"dsub{h}")
                        nc.vector.tensor_sub(dsub, repS[1][h], repS[2][h])
                        nc.scalar.activation(dist[h], dsub, AF.Tanh)
                    op = hps.tile([OUT, GC], f32, tag="op")
                    for h in (0, 1):
                        nc.tensor.matmul(op, predw[:, h, :], dist[h],
                                         start=(h == 0), stop=(h == 1))
                    oS = hp.tile([OUT, GC], f32, tag="oS")
                    nc.scalar.activation(oS, op, AF.Identity, bias=predb)
                    nc.sync.dma_start(out=out_dram[:, :], in_=oS)

    nc.finalize()
    return nc


# ----------------------------------------------------------------------------
# Host-side preparation
# ----------------------------------------------------------------------------

def _perm_gates():
    """Columns: pytorch gate order [i,f,g,o] -> slots [i0,i1,f0,f1,o0,o1,g0,g1]."""
    idx = np.arange(H4).reshape(4, 2, 128)
    order = [(0, 0), (0, 1), (1, 0), (1, 1), (3, 0), (3, 1), (2, 0), (2, 1)]
    return np.concatenate([idx[g, h] for g, h in order])


def _prep_lstm(lstm, perm):
    out = {}
    for layer in range(2):
        for d in range(2):
            Wih, Whh, bih, bhh = [np.asarray(a) for a in lstm[layer][d]]
            bias = (bih + bhh)[perm]
            WihT = Wih.T[:, perm]
            WhhT = Whh.T[:, perm]
            if layer == 0:
                WihT = np.concatenate([WihT, bias[None, :]], 0)
                out[f"w1ih{d}"] = np.ascontiguousarray(WihT).astype(BF16)
                out[f"w1hh{d}"] = np.ascontiguousarray(WhhT).astype(BF16)
            else:
                out[f"w2ih{d}"] = np.ascontiguousarray(WihT).astype(BF16)
                out[f"w2hh{d}"] = np.ascontiguousarray(WhhT).astype(BF16)
                out[f"b2_{d}"] = bias[None, :].astype(BF16)
    return out


def _prep_adj(src, dst):
    src = np.asarray(src).astype(np.int64)
    dst = np.asarray(dst).astype(np.int64)
    g = dst // NPG
    sl = src - g * NPG
    dl = dst - g * NPG
    flat = (g * NPG + dl) * NPG + sl
    A = np.bincount(flat, minlength=G * NPG * NPG).reshape(G, NPG, NPG)
    outdeg = np.maximum(A.sum(axis=1), 1.0)
    Ahat = A / np.sqrt(8.0) / np.sqrt(outdeg)[:, None, :]
    return Ahat.astype(np.float32)


def _pack_adj_core(Ahat_c):
    """[GC,35,35] -> [106, NBLK*105] bf16 lhsT blocks with bias-ones row.

    lhsT block layout: row k = source node in the projection's M order, which
    is t-major (k = t_src*3 + i for graph position i), plus row 105 = bias
    ones; col m = destination node position-major (m = i*35 + t_dst), to
    match the pooling matrix.  Block NBLK-1 covers graphs GC-3..GC-1 but only
    position 2 (graph GC-1) is live."""
    out = np.zeros((105, NBLK * 105), np.float32)
    for blk in range(NBLK):
        if blk < NBLK - 1:
            g0, live = blk * 3, (0, 1, 2)
        else:
            g0, live = GC - 3, (2,)
        for i in live:
            c0 = blk * 105 + i * NPG
            # rows t_src*3 + i, cols t_dst: Ahat[t_dst, t_src].T = [src, dst]
            out[i:i + 3 * NPG:3, c0:c0 + NPG] = Ahat_c[g0 + i].T
    return out.astype(BF16)


def _prep_host(x1, x2, src1, dst1, src2, dst2, lstm1, lstm2,
               gcn1_w, gcn1_b, gcn2_w, gcn2_b, pred_w, pred_b):
    perm = _perm_gates()
    branches = {}
    for br, (x, lstm, gw_, gb_, src, dst) in enumerate(
            [(x1, lstm1, gcn1_w, gcn1_b, src1, dst1),
             (x2, lstm2, gcn2_w, gcn2_b, src2, dst2)], start=1):
        branches[br] = (np.asarray(x, np.float32), _prep_lstm(lstm, perm),
                        np.asarray(gw_), np.asarray(gb_),
                        _prep_adj(src, dst))

    pool3 = np.zeros((105, 3), np.float32)
    for i in range(3):
        pool3[i * NPG:(i + 1) * NPG, i] = 1.0 / NPG

    in_maps = []
    for c in range(NCORES):
        m = {}
        for br in (1, 2):
            x, lw, gw_, gb_, Ahat = branches[br]
            xs = x[c * B:(c + 1) * B]
            xT = np.empty((INPUT + 1, B), np.float32)
            xT[:INPUT] = xs.reshape(GC, NPG, INPUT).transpose(2, 1, 0) \
                           .reshape(INPUT, B)
            xT[INPUT] = 1.0
            m[f"xT{br}"] = xT.astype(BF16)
            for d in (0, 1):
                for k in ("w1ih", "w1hh", "w2ih", "w2hh"):
                    m[f"{k}{br}{d}"] = lw[f"{k}{d}"]
                m[f"b2_{br}{d}"] = lw[f"b2_{d}"]
            m[f"adjT{br}"] = _pack_adj_core(Ahat[c * GC:(c + 1) * GC])
            m[f"gw{br}"] = gw_.astype(BF16)
            m[f"gb{br}"] = np.broadcast_to(
            gb_.reshape(1, HG), (105, HG)).astype(BF16)
        m["predw"] = np.asarray(pred_w).astype(BF16)
        m["predb"] = np.asarray(pred_b).reshape(OUT, 1).astype(np.float32)
        m["pool3"] = pool3.astype(BF16)
        in_maps.append(m)
    return in_maps


def kernel(**inputs):
    from concourse.bass_utils import run_bass_kernel_spmd

    in_maps = _prep_host(**inputs)
    if "nc" not in _CACHE:
        _CACHE["nc"] = _build_bass()
    res = run_bass_kernel_spmd(_CACHE["nc"], in_maps,
                               core_ids=list(range(NCORES)))
    out = np.empty((G, OUT), np.float32)
    for c in range(NCORES):
        out[c * GC:(c + 1) * GC] = res.results[c]["outT"].T
    return out
